# revision 2
# baseline (speedup 1.0000x reference)
"""GridRNN (2D recurrence) Trainium2 Bass kernel.

Sharding: data-parallel over batch (B=8 -> 8 cores, zero collectives).
Per core: 3D wavefront over (depth, anti-diagonal): step (d, t) runs at
wavefront tick t+d, so the PE interleaves matmuls of all three depths and
stays busy while ACT/DVE finish the previous diagonal of a given depth.
State is kept pre-transposed ([2H, R] feature-major slabs, zero-padded
columns at both ends so i=0 / j=0 boundary states are free). Per step:
  - 8 projection matmuls (input projection accumulated straight into the
    recurrence PSUM; depth d>0 reads depth d-1's slab produced this tick)
  - 16 recurrence matmuls, 2-way column-tiled: hx-half -> PSUM rows 0:R
    (tile pos (0,0)), hy-half -> PSUM rows 64:64+R ((0,64)) so both
    halves stream concurrently through separate PE column groups
  - tanh on ACT -> h [R, 1024] in SBUF, DMA out
  - 8 PE-transposes of h packed into one PSUM bank -> next slabs
"""
import sys
sys.path.insert(0, "/opt/trn_rl_repo")
import numpy as np
import concourse.bass as bass
import concourse.tile as tile
from concourse import bacc, mybir
from concourse import bass_utils

FP32 = mybir.dt.float32
B, I, J, H, D = 8, 48, 48, 512, 3
H2 = 2 * H
NK2, NK1 = H2 // 128, H // 128  # 8, 4
ND = I + J - 1                  # 95
TANH = mybir.ActivationFunctionType.Tanh

_cache = {}


def _build(has_bias: bool):
    nc = bacc.Bacc("TRN2", target_bir_lowering=False, debug=False, num_devices=B)
    srcT_d = nc.dram_tensor("srcT", [H, I], FP32, kind="ExternalInput")
    trgTr_d = nc.dram_tensor("trgTr", [H, J], FP32, kind="ExternalInput")
    wxh_d = nc.dram_tensor("wxh", [D, NK2, 128, H], FP32, kind="ExternalInput")
    wyh_d = nc.dram_tensor("wyh", [D, NK2, 128, H], FP32, kind="ExternalInput")
    wxi_d = nc.dram_tensor("wxi", [D, NK1, 128, H], FP32, kind="ExternalInput")
    wyi_d = nc.dram_tensor("wyi", [D, NK1, 128, H], FP32, kind="ExternalInput")
    idn_d = nc.dram_tensor("idn", [128, 128], FP32, kind="ExternalInput")
    if has_bias:
        bsx_d = nc.dram_tensor("bsx", [D, H], FP32, kind="ExternalInput")
        bsy_d = nc.dram_tensor("bsy", [D, H], FP32, kind="ExternalInput")
        ones_d = nc.dram_tensor("ones", [1, I], FP32, kind="ExternalInput")
    out_d = nc.dram_tensor("out", [D, I, J, 2, H], FP32, kind="ExternalOutput")

    outv = out_d.ap().rearrange("dp i j two h -> dp (i j) two h")

    def diag_geom(t):
        i_lo = max(0, t - (J - 1))
        i_hi = min(t, I - 1)
        return i_lo, i_hi - i_lo + 1

    with tile.TileContext(nc) as tc:
        with (
            tc.tile_pool(name="const", bufs=1) as constp,
            tc.tile_pool(name="wp", bufs=1) as wp,
            tc.tile_pool(name="slab", bufs=9) as slabp,
            tc.tile_pool(name="hsb", bufs=4) as hsbp,
            tc.tile_pool(name="pre", bufs=6, space="PSUM") as prep,
            tc.tile_pool(name="tpp", bufs=2, space="PSUM") as tpp,
        ):
            idn = constp.tile([128, 128], FP32, tag="idn")
            nc.sync.dma_start(idn[:], idn_d.ap())
            srcT = constp.tile([128, NK1, I], FP32, tag="srcT")
            nc.sync.dma_start(srcT[:], srcT_d.ap().rearrange("(c p) i -> p c i", p=128))
            trgTr = constp.tile([128, NK1, J], FP32, tag="trgTr")
            nc.sync.dma_start(trgTr[:], trgTr_d.ap().rearrange("(c p) j -> p c j", p=128))
            if has_bias:
                ones = constp.tile([1, I], FP32, tag="ones")
                nc.sync.dma_start(ones[:], ones_d.ap())

            wxh, wyh, wxi, wyi, bsx, bsy = [], [], [], [], [], []
            for d in range(D):
                for lst, dram, nk, nm in ((wxh, wxh_d, NK2, "wxh"),
                                          (wyh, wyh_d, NK2, "wyh"),
                                          (wxi, wxi_d, NK1, "wxi"),
                                          (wyi, wyi_d, NK1, "wyi")):
                    w = wp.tile([128, nk, H], FP32, tag=f"{nm}{d}")
                    nc.sync.dma_start(w[:], dram.ap().rearrange("d c p n -> d p c n")[d])
                    lst.append(w)
                if has_bias:
                    bx = wp.tile([1, H], FP32, tag=f"bsx{d}")
                    nc.sync.dma_start(bx[:], bsx_d.ap()[d:d + 1, :])
                    bsx.append(bx)
                    by = wp.tile([1, H], FP32, tag=f"bsy{d}")
                    nc.sync.dma_start(by[:], bsy_d.ap()[d:d + 1, :])
                    bsy.append(by)

            # per-depth rolling state
            hxp, hyp = [None] * D, [None] * D      # slabs of previous diag
            cur = [None] * D                       # slabs of current diag
            pre_next = [None] * D                  # psum of next diag
            for d in range(D):
                hx0 = slabp.tile([128, NK1, I + 2], FP32, tag="hx")
                hy0 = slabp.tile([128, NK1, I + 2], FP32, tag="hy")
                nc.vector.memset(hx0[:], 0.0)
                nc.vector.memset(hy0[:], 0.0)
                hxp[d], hyp[d] = hx0, hy0

            def emit_proj(d, t):
                """Projection matmuls for (d, t) into a fresh psum tile.
                hx-half -> rows [0:R] (col group 0), hy-half -> rows
                [64:64+R] (col group 1); both accumulate ahead of the
                recurrence so the PE has independent work during tanh."""
                i_lo, R = diag_geom(t)
                pre = prep.tile([128, H], FP32, tag="pre")
                if d == 0:
                    j0 = (J - 1) - t + i_lo
                    for k in range(NK1):
                        nc.tensor.matmul(pre[0:R, :], srcT[:, k, i_lo:i_lo + R],
                                         wxi[0][:, k, :], start=(k == 0), stop=False)
                        nc.tensor.matmul(pre[64:64 + R, :], trgTr[:, k, j0:j0 + R],
                                         wyi[0][:, k, :], start=(k == 0), stop=False)
                else:
                    sx, sy = cur[d - 1]  # depth d-1 slabs of this same diag t
                    for k in range(NK1):
                        nc.tensor.matmul(pre[0:R, :], sx[:, k, 1:1 + R],
                                         wxi[d][:, k, :], start=(k == 0), stop=False)
                        nc.tensor.matmul(pre[64:64 + R, :], sy[:, k, 1:1 + R],
                                         wyi[d][:, k, :], start=(k == 0), stop=False)
                if has_bias:
                    nc.tensor.matmul(pre[0:R, :], ones[0:1, 0:R], bsx[d][:],
                                     start=False, stop=False)
                    nc.tensor.matmul(pre[64:64 + R, :], ones[0:1, 0:R], bsy[d][:],
                                     start=False, stop=False)
                return pre

            pre_next[0] = emit_proj(0, 0)

            def step(d, t):
                i_lo, R = diag_geom(t)
                off = i_lo - max(0, t - J)
                pre = pre_next[d]
                for k in range(NK2):
                    if k < NK1:
                        st = hxp[d][:, k, off:off + R]
                    else:
                        st = hyp[d][:, k - NK1, off + 1:off + 1 + R]
                    last = (k == NK2 - 1)
                    nc.tensor.matmul(pre[0:R, :], st, wxh[d][:, k, :],
                                     start=False, stop=last)
                    nc.tensor.matmul(pre[64:64 + R, :], st, wyh[d][:, k, :],
                                     start=False, stop=last)

                h = hsbp.tile([I, H2], FP32, tag="h")
                nc.scalar.activation(h[0:R, 0:H], pre[0:R, :], TANH)
                nc.scalar.activation(h[0:R, H:H2], pre[64:64 + R, :], TANH)

                row0 = i_lo * (J - 1) + t
                sl = slice(row0, row0 + (J - 1) * (R - 1) + 1, J - 1)
                nc.sync.dma_start(outv[d, sl, 0, :], h[0:R, 0:H])
                nc.sync.dma_start(outv[d, sl, 1, :], h[0:R, H:H2])

                tp = tpp.tile([128, 512], FP32, tag="tp")
                for k in range(NK2):
                    nc.tensor.transpose(tp[:, 64 * k:64 * k + R],
                                        h[0:R, 128 * k:128 * (k + 1)],
                                        idn[0:R, 0:R])
                hxn = slabp.tile([128, NK1, I + 2], FP32, tag="hx")
                hyn = slabp.tile([128, NK1, I + 2], FP32, tag="hy")
                tpv = tp[:].rearrange("p (c w) -> p c w", w=64)
                nc.vector.tensor_copy(hxn[:, :, 1:R + 1], tpv[:, 0:NK1, 0:R])
                nc.vector.tensor_copy(hyn[:, :, 1:R + 1], tpv[:, NK1:NK2, 0:R])
                nc.vector.memset(hxn[:, :, 0:1], 0.0)
                nc.vector.memset(hxn[:, :, R + 1:R + 2], 0.0)
                nc.vector.memset(hyn[:, :, 0:1], 0.0)
                nc.vector.memset(hyn[:, :, R + 1:R + 2], 0.0)
                cur[d] = (hxn, hyn)
                hxp[d], hyp[d] = hxn, hyn

                # projections that become runnable after this step:
                if t + 1 < ND:
                    pre_next[d] = emit_proj(d, t + 1)  # (uses cur[d-1] @ t+1)
                if t == 0 and d + 1 < D:
                    pre_next[d + 1] = emit_proj(d + 1, 0)  # uses this step's slabs

            for tw in range(ND + D - 1):
                for d in range(D):
                    t = tw - d
                    if 0 <= t < ND:
                        step(d, t)
    nc.compile()
    return nc


def kernel(**inputs):
    src = np.ascontiguousarray(np.asarray(inputs["src_seq_batch"], dtype=np.float32))
    trg = np.ascontiguousarray(np.asarray(inputs["trg_seq_batch"], dtype=np.float32))
    Wx_ih = np.asarray(inputs["Wx_ih"], dtype=np.float32)
    Wx_hh = np.asarray(inputs["Wx_hh"], dtype=np.float32)
    Wy_ih = np.asarray(inputs["Wy_ih"], dtype=np.float32)
    Wy_hh = np.asarray(inputs["Wy_hh"], dtype=np.float32)
    bsx = (np.asarray(inputs["bx_ih"], np.float32)
           + np.asarray(inputs["bx_hh"], np.float32))
    bsy = (np.asarray(inputs["by_ih"], np.float32)
           + np.asarray(inputs["by_hh"], np.float32))
    depth = int(np.asarray(inputs["depth"]))
    assert depth == D and src.shape == (B, I, H) and trg.shape == (B, J, H)
    has_bias = bool(np.any(bsx) or np.any(bsy))

    if has_bias not in _cache:
        _cache[has_bias] = _build(has_bias)
    nc = _cache[has_bias]

    wxh = np.ascontiguousarray(Wx_hh.reshape(D, NK2, 128, H))
    wyh = np.ascontiguousarray(Wy_hh.reshape(D, NK2, 128, H))
    wxi = np.ascontiguousarray(Wx_ih.reshape(D, NK1, 128, H))
    wyi = np.ascontiguousarray(Wy_ih.reshape(D, NK1, 128, H))
    idn = np.eye(128, dtype=np.float32)

    in_maps = []
    for c in range(B):
        m = {
            "srcT": np.ascontiguousarray(src[c].T),
            "trgTr": np.ascontiguousarray(trg[c].T[:, ::-1]),
            "wxh": wxh, "wyh": wyh, "wxi": wxi, "wyi": wyi, "idn": idn,
        }
        if has_bias:
            m["bsx"] = np.ascontiguousarray(bsx)
            m["bsy"] = np.ascontiguousarray(bsy)
            m["ones"] = np.ones((1, I), dtype=np.float32)
        in_maps.append(m)

    res = bass_utils.run_bass_kernel_spmd(nc, in_maps, list(range(B)))
    return np.stack([res.results[c]["out"] for c in range(B)], axis=0)


# revision 3
# speedup vs baseline: 3.7725x; 3.7725x over previous
"""GridRNN (2D recurrence) Trainium2 Bass kernel.

Sharding: data-parallel over batch (B=8 -> 8 cores, zero collectives).
Per core: 3D wavefront over (depth, anti-diagonal): step (d, t) runs at
wavefront tick t+d, so the PE interleaves matmuls of all three depths and
stays busy while ACT/DVE finish the previous diagonal of a given depth.
State is kept pre-transposed ([2H, R] feature-major slabs, zero-padded
columns at both ends so i=0 / j=0 boundary states are free). Per step:
  - 8 projection matmuls (input projection accumulated straight into the
    recurrence PSUM; depth d>0 reads depth d-1's slab produced this tick)
  - 16 recurrence matmuls, 2-way column-tiled: hx-half -> PSUM rows 0:R
    (tile pos (0,0)), hy-half -> PSUM rows 64:64+R ((0,64)) so both
    halves stream concurrently through separate PE column groups
  - tanh on ACT -> h [R, 1024] in SBUF, DMA out
  - 8 PE-transposes of h packed into one PSUM bank -> next slabs
"""
import sys
sys.path.insert(0, "/opt/trn_rl_repo")
import numpy as np
import concourse.bass as bass
import concourse.tile as tile
from concourse import bacc, mybir
from concourse import bass_utils

FP32 = mybir.dt.float32
FP32R = mybir.dt.float32r

def _r(ap):
    return ap.bitcast(FP32R)
B, I, J, H, D = 8, 48, 48, 512, 3
H2 = 2 * H
NK2, NK1 = H2 // 128, H // 128  # 8, 4
ND = I + J - 1                  # 95
TANH = mybir.ActivationFunctionType.Tanh

_cache = {}


def _build(has_bias: bool):
    nc = bacc.Bacc("TRN2", target_bir_lowering=False, debug=False, num_devices=B)
    srcT_d = nc.dram_tensor("srcT", [H, I], FP32, kind="ExternalInput")
    trgTr_d = nc.dram_tensor("trgTr", [H, J], FP32, kind="ExternalInput")
    wxh_d = nc.dram_tensor("wxh", [D, NK2, 128, H], FP32, kind="ExternalInput")
    wyh_d = nc.dram_tensor("wyh", [D, NK2, 128, H], FP32, kind="ExternalInput")
    wxi_d = nc.dram_tensor("wxi", [D, NK1, 128, H], FP32, kind="ExternalInput")
    wyi_d = nc.dram_tensor("wyi", [D, NK1, 128, H], FP32, kind="ExternalInput")
    idn_d = nc.dram_tensor("idn", [128, 128], FP32, kind="ExternalInput")
    if has_bias:
        bsx_d = nc.dram_tensor("bsx", [D, H], FP32, kind="ExternalInput")
        bsy_d = nc.dram_tensor("bsy", [D, H], FP32, kind="ExternalInput")
        ones_d = nc.dram_tensor("ones", [1, I], FP32, kind="ExternalInput")
    out_d = nc.dram_tensor("out", [D, I, J, 2, H], FP32, kind="ExternalOutput")

    outv = out_d.ap().rearrange("dp i j two h -> dp (i j) two h")

    def diag_geom(t):
        i_lo = max(0, t - (J - 1))
        i_hi = min(t, I - 1)
        return i_lo, i_hi - i_lo + 1

    with tile.TileContext(nc) as tc:
        with (
            tc.tile_pool(name="const", bufs=1) as constp,
            tc.tile_pool(name="wp", bufs=1) as wp,
            tc.tile_pool(name="slab", bufs=9) as slabp,
            tc.tile_pool(name="hsb", bufs=4) as hsbp,
            tc.tile_pool(name="pre", bufs=6, space="PSUM") as prep,
            tc.tile_pool(name="tpp", bufs=2, space="PSUM") as tpp,
        ):
            idn = constp.tile([128, 128], FP32, tag="idn")
            nc.sync.dma_start(idn[:], idn_d.ap())
            srcT = constp.tile([128, NK1, I], FP32, tag="srcT")
            nc.sync.dma_start(srcT[:], srcT_d.ap().rearrange("(c p) i -> p c i", p=128))
            trgTr = constp.tile([128, NK1, J], FP32, tag="trgTr")
            nc.sync.dma_start(trgTr[:], trgTr_d.ap().rearrange("(c p) j -> p c j", p=128))
            if has_bias:
                ones = constp.tile([1, I], FP32, tag="ones")
                nc.sync.dma_start(ones[:], ones_d.ap())

            wxh, wyh, wxi, wyi, bsx, bsy = [], [], [], [], [], []
            for d in range(D):
                for lst, dram, nk, nm in ((wxh, wxh_d, NK2, "wxh"),
                                          (wyh, wyh_d, NK2, "wyh"),
                                          (wxi, wxi_d, NK1, "wxi"),
                                          (wyi, wyi_d, NK1, "wyi")):
                    w = wp.tile([128, nk, H], FP32, tag=f"{nm}{d}")
                    nc.sync.dma_start(w[:], dram.ap().rearrange("d c p n -> d p c n")[d])
                    lst.append(w)
                if has_bias:
                    bx = wp.tile([1, H], FP32, tag=f"bsx{d}")
                    nc.sync.dma_start(bx[:], bsx_d.ap()[d:d + 1, :])
                    bsx.append(bx)
                    by = wp.tile([1, H], FP32, tag=f"bsy{d}")
                    nc.sync.dma_start(by[:], bsy_d.ap()[d:d + 1, :])
                    bsy.append(by)

            # per-depth rolling state
            hxp, hyp = [None] * D, [None] * D      # slabs of previous diag
            cur = [None] * D                       # slabs of current diag
            pre_next = [None] * D                  # psum of next diag
            for d in range(D):
                hx0 = slabp.tile([128, NK1, I + 2], FP32, tag="hx")
                hy0 = slabp.tile([128, NK1, I + 2], FP32, tag="hy")
                nc.vector.memset(hx0[:], 0.0)
                nc.vector.memset(hy0[:], 0.0)
                hxp[d], hyp[d] = hx0, hy0

            def emit_proj(d, t):
                """Projection matmuls for (d, t) into a fresh psum tile.
                hx-half -> rows [0:R] (col group 0), hy-half -> rows
                [64:64+R] (col group 1); both accumulate ahead of the
                recurrence so the PE has independent work during tanh."""
                i_lo, R = diag_geom(t)
                pre = prep.tile([128, H], FP32, tag="pre")
                if d == 0:
                    j0 = (J - 1) - t + i_lo
                    for k in range(NK1):
                        nc.tensor.matmul(pre[0:R, :], _r(srcT[:, k, i_lo:i_lo + R]),
                                         _r(wxi[0][:, k, :]), start=(k == 0), stop=False)
                        nc.tensor.matmul(pre[64:64 + R, :], _r(trgTr[:, k, j0:j0 + R]),
                                         _r(wyi[0][:, k, :]), start=(k == 0), stop=False)
                else:
                    sx, sy = cur[d - 1]  # depth d-1 slabs of this same diag t
                    for k in range(NK1):
                        nc.tensor.matmul(pre[0:R, :], _r(sx[:, k, 1:1 + R]),
                                         _r(wxi[d][:, k, :]), start=(k == 0), stop=False)
                        nc.tensor.matmul(pre[64:64 + R, :], _r(sy[:, k, 1:1 + R]),
                                         _r(wyi[d][:, k, :]), start=(k == 0), stop=False)
                if has_bias:
                    nc.tensor.matmul(pre[0:R, :], _r(ones[0:1, 0:R]), _r(bsx[d][:]),
                                     start=False, stop=False)
                    nc.tensor.matmul(pre[64:64 + R, :], _r(ones[0:1, 0:R]), _r(bsy[d][:]),
                                     start=False, stop=False)
                return pre

            pre_next[0] = emit_proj(0, 0)

            def step(d, t):
                i_lo, R = diag_geom(t)
                off = i_lo - max(0, t - J)
                pre = pre_next[d]
                for k in range(NK2):
                    if k < NK1:
                        st = hxp[d][:, k, off:off + R]
                    else:
                        st = hyp[d][:, k - NK1, off + 1:off + 1 + R]
                    last = (k == NK2 - 1)
                    nc.tensor.matmul(pre[0:R, :], _r(st), _r(wxh[d][:, k, :]),
                                     start=False, stop=last)
                    nc.tensor.matmul(pre[64:64 + R, :], _r(st), _r(wyh[d][:, k, :]),
                                     start=False, stop=last)

                h = hsbp.tile([I, H2], FP32, tag="h")
                nc.scalar.activation(h[0:R, 0:H], pre[0:R, :], TANH)
                nc.scalar.activation(h[0:R, H:H2], pre[64:64 + R, :], TANH)

                row0 = i_lo * (J - 1) + t
                sl = slice(row0, row0 + (J - 1) * (R - 1) + 1, J - 1)
                nc.sync.dma_start(outv[d, sl, 0, :], h[0:R, 0:H])
                nc.sync.dma_start(outv[d, sl, 1, :], h[0:R, H:H2])

                tp = tpp.tile([128, 512], FP32, tag="tp")
                for k in range(NK2):
                    nc.tensor.transpose(tp[:, 64 * k:64 * k + R],
                                        h[0:R, 128 * k:128 * (k + 1)],
                                        idn[0:R, 0:R])
                hxn = slabp.tile([128, NK1, I + 2], FP32, tag="hx")
                hyn = slabp.tile([128, NK1, I + 2], FP32, tag="hy")
                tpv = tp[:].rearrange("p (c w) -> p c w", w=64)
                nc.vector.tensor_copy(hxn[:, :, 1:R + 1], tpv[:, 0:NK1, 0:R])
                nc.vector.tensor_copy(hyn[:, :, 1:R + 1], tpv[:, NK1:NK2, 0:R])
                nc.vector.memset(hxn[:, :, 0:1], 0.0)
                nc.vector.memset(hxn[:, :, R + 1:R + 2], 0.0)
                nc.vector.memset(hyn[:, :, 0:1], 0.0)
                nc.vector.memset(hyn[:, :, R + 1:R + 2], 0.0)
                cur[d] = (hxn, hyn)
                hxp[d], hyp[d] = hxn, hyn

                # projections that become runnable after this step:
                if t + 1 < ND:
                    pre_next[d] = emit_proj(d, t + 1)  # (uses cur[d-1] @ t+1)
                if t == 0 and d + 1 < D:
                    pre_next[d + 1] = emit_proj(d + 1, 0)  # uses this step's slabs

            for tw in range(ND + D - 1):
                for d in range(D):
                    t = tw - d
                    if 0 <= t < ND:
                        step(d, t)
    nc.compile()
    return nc


def kernel(**inputs):
    src = np.ascontiguousarray(np.asarray(inputs["src_seq_batch"], dtype=np.float32))
    trg = np.ascontiguousarray(np.asarray(inputs["trg_seq_batch"], dtype=np.float32))
    Wx_ih = np.asarray(inputs["Wx_ih"], dtype=np.float32)
    Wx_hh = np.asarray(inputs["Wx_hh"], dtype=np.float32)
    Wy_ih = np.asarray(inputs["Wy_ih"], dtype=np.float32)
    Wy_hh = np.asarray(inputs["Wy_hh"], dtype=np.float32)
    bsx = (np.asarray(inputs["bx_ih"], np.float32)
           + np.asarray(inputs["bx_hh"], np.float32))
    bsy = (np.asarray(inputs["by_ih"], np.float32)
           + np.asarray(inputs["by_hh"], np.float32))
    depth = int(np.asarray(inputs["depth"]))
    assert depth == D and src.shape == (B, I, H) and trg.shape == (B, J, H)
    has_bias = bool(np.any(bsx) or np.any(bsy))

    if has_bias not in _cache:
        _cache[has_bias] = _build(has_bias)
    nc = _cache[has_bias]

    wxh = np.ascontiguousarray(Wx_hh.reshape(D, NK2, 128, H))
    wyh = np.ascontiguousarray(Wy_hh.reshape(D, NK2, 128, H))
    wxi = np.ascontiguousarray(Wx_ih.reshape(D, NK1, 128, H))
    wyi = np.ascontiguousarray(Wy_ih.reshape(D, NK1, 128, H))
    idn = np.eye(128, dtype=np.float32)

    in_maps = []
    for c in range(B):
        m = {
            "srcT": np.ascontiguousarray(src[c].T),
            "trgTr": np.ascontiguousarray(trg[c].T[:, ::-1]),
            "wxh": wxh, "wyh": wyh, "wxi": wxi, "wyi": wyi, "idn": idn,
        }
        if has_bias:
            m["bsx"] = np.ascontiguousarray(bsx)
            m["bsy"] = np.ascontiguousarray(bsy)
            m["ones"] = np.ones((1, I), dtype=np.float32)
        in_maps.append(m)

    res = bass_utils.run_bass_kernel_spmd(nc, in_maps, list(range(B)))
    return np.stack([res.results[c]["out"] for c in range(B)], axis=0)
